# revision 5
# baseline (speedup 1.0000x reference)
"""MMoE-style CustomizedGateControl kernel for 8x TRN2 NeuronCores.

Data-parallel over the batch dim (16384 -> 8 x 2048). Per core, expert
GEMMs run weights-stationary streaming batch columns, producing outputs
directly in [e, b] layout:
  - 12 expert GEMMs as 24 column-blocks (eb) of 128 e-rows x 2048 b-cols,
    PSUM [128, 2048] (4 banks) double-buffered
  - drain = ONE fused ACT op per eb: relu(psum + bias[e]) -> fp16 SBUF
    (bias is per-partition in this layout)
  - gates [16, 2048] via tiny PE GEMM, broadcast to [128, 16, 2048] with
    partition-replicating DMA reads from a DRAM bounce buffer
  - gated combine on DVE + Pool: acc_t += X_eb * gbc[t,g] (fp16 2x mode)
  - tower MLPs read the accumulators directly (no transpose needed)
All parameters replicated; no collectives.
"""

import sys

if "/opt/trn_rl_repo" not in sys.path:
    sys.path.insert(0, "/opt/trn_rl_repo")

import numpy as np

import concourse.bacc as bacc
import concourse.mybir as mybir
import concourse.tile as tile
from concourse.bass_utils import run_bass_kernel_spmd

# problem dims
B, D, E, H = 16384, 512, 256, 128
S, K, T = 4, 4, 2
NCORES = 8
BC = B // NCORES          # 2048 batch rows per core
P = 128                   # partitions
KC = D // P               # 4 contraction chunks
NE = S + T * K            # 12 experts
G = S + K                 # 8 gate inputs per task
TG = T * G                # 16 gate columns
NEB = NE * 2              # 24 expert-output blocks of 128 e-rows
WCOLS = NE * E            # 3072 expert output columns
WALL = WCOLS + TG         # 3088 = experts + gate columns
NBK = BC // 512           # 4 batch chunks of 512 (psum bank size)

f32 = mybir.dt.float32
f16 = mybir.dt.float16

# sweep order: all half-0 blocks first, then half-1 (logical eb = expert*2+h)
SWEEP = [e * 2 for e in range(NE)] + [e * 2 + 1 for e in range(NE)]


def _uses(expert: int):
    """(task, gate-col j) pairs consuming this expert's output."""
    if expert < S:
        return [(t, t * G + expert) for t in range(T)]
    t = (expert - S) // K
    return [(t, t * G + S + (expert - S) % K)]


def _build():
    nc = bacc.Bacc("TRN2", target_bir_lowering=False, debug=False)

    xt_d = nc.dram_tensor("xt", [D, BC], f16, kind="ExternalInput").ap()
    wall_d = nc.dram_tensor("wall", [D, WALL], f16, kind="ExternalInput").ap()
    biasT_d = nc.dram_tensor("biasT", [P, NEB], f32, kind="ExternalInput").ap()
    tw1_d = nc.dram_tensor("tw1", [T, E, H], f16, kind="ExternalInput").ap()
    tb1_d = nc.dram_tensor("tb1", [H, T], f32, kind="ExternalInput").ap()
    tw2_d = nc.dram_tensor("tw2", [H, T], f16, kind="ExternalInput").ap()
    out_d = nc.dram_tensor("out", [T, BC], f32, kind="ExternalOutput").ap()
    gbounce_d = nc.dram_tensor("gbounce", [TG, BC], f16, kind="Internal").ap()

    with tile.TileContext(nc) as tc:
        with (
            tc.tile_pool(name="const", bufs=1) as const,
            tc.tile_pool(name="exp", bufs=6) as exp_pool,
            tc.tile_pool(name="tmpv", bufs=2) as tmpv_pool,
            tc.tile_pool(name="tmpg", bufs=2) as tmpg_pool,
            tc.tile_pool(name="hsb", bufs=2) as hsb_pool,
        ):
            xt_t = [const.tile([P, BC], f16, tag=f"xt{k}", name=f"xt{k}") for k in range(KC)]
            wall_t = [const.tile([P, WALL], f16, tag=f"wall{k}", name=f"wall{k}") for k in range(KC)]
            biasT = const.tile([P, NEB], f32, tag="biasT", name="biasT")
            gates_sb = const.tile([TG, BC], f16, tag="gates_sb", name="gates_sb")
            gbc = const.tile([P, TG, BC], f16, tag="gbc", name="gbc")
            acc = [
                const.tile([P, BC], f16, tag=f"acc{i}", name=f"acc{i}") for i in range(T * 2)
            ]
            tw1_t = {}
            for t in range(T):
                for kc in range(2):
                    t_ = const.tile([P, H], f16, tag=f"tw1_{t}_{kc}", name=f"tw1_{t}_{kc}")
                    tw1_t[(t, kc)] = t_
            tb1 = const.tile([H, T], f32, tag="tb1", name="tb1")
            tw2 = const.tile([H, T], f16, tag="tw2", name="tw2")
            out_sb = const.tile([1, T * BC], f32, tag="out_sb", name="out_sb")

            # ---- input DMAs ----
            # xt: sync gets k=0,1; scalar k=2,3 (gates need all of xt first)
            nc.sync.dma_start(xt_t[0][:], xt_d[0 * P : 1 * P, :])
            nc.scalar.dma_start(xt_t[2][:], xt_d[2 * P : 3 * P, :])
            nc.sync.dma_start(xt_t[1][:], xt_d[1 * P : 2 * P, :])
            nc.scalar.dma_start(xt_t[3][:], xt_d[3 * P : 4 * P, :])
            # gpsimd: gate weight cols first, then wall halves in sweep order
            for k in range(KC):
                rs = slice(k * P, (k + 1) * P)
                nc.gpsimd.dma_start(wall_t[k][:, WCOLS:WALL], wall_d[rs, WCOLS:WALL])
            HALF = NEB // 2 * P  # 1536 cols per half
            for k in range(KC):
                rs = slice(k * P, (k + 1) * P)
                nc.gpsimd.dma_start(wall_t[k][:, 0:HALF], wall_d[rs, 0:HALF])
            for k in range(KC):
                rs = slice(k * P, (k + 1) * P)
                nc.gpsimd.dma_start(wall_t[k][:, HALF:WCOLS], wall_d[rs, HALF:WCOLS])
            # scalar: small consts
            nc.scalar.dma_start(biasT[:], biasT_d[:])
            nc.scalar.dma_start(tb1[:], tb1_d[:])
            nc.scalar.dma_start(tw2[:], tw2_d[:])
            for t in range(T):
                for kc in range(2):
                    nc.scalar.dma_start(
                        tw1_t[(t, kc)][:], tw1_d[t, kc * P : (kc + 1) * P, :]
                    )

            # gates first: [16, 2048] = gw.T @ xt
            with tc.tile_pool(name="gateps", bufs=1, space="PSUM") as gateps_pool:
                gate_ps = gateps_pool.tile([TG, BC], f32, tag="gateps", name="gateps")
                for j in range(NBK):
                    cs = slice(j * 512, (j + 1) * 512)
                    for k in range(KC):
                        nc.tensor.matmul(
                            gate_ps[:, cs],
                            wall_t[k][:, WCOLS:WALL],
                            xt_t[k][:, cs],
                            start=(k == 0),
                            stop=(k == KC - 1),
                        )
                nc.scalar.copy(gates_sb[:], gate_ps[:])

            # bounce gates through DRAM with partition-replicating reads.
            # Each queue writes its own row-slice then reads it back
            # (same-queue FIFO ordering guarantees write-before-read).
            row_groups = [
                (nc.sync, [0, 8, 1, 9, 2, 10]),
                (nc.scalar, [3, 11, 4, 5]),
                (nc.gpsimd, [6, 7, 12, 13, 14, 15]),
            ]
            for eng, rows in row_groups:
                for r in rows:
                    eng.dma_start(gbounce_d[r : r + 1, :], gates_sb[r : r + 1, :])
                for r in rows:
                    eng.dma_start(
                        gbc[:, r, :], gbounce_d[r : r + 1, :].broadcast_to([P, BC])
                    )

            with tc.tile_pool(name="expps", bufs=2, space="PSUM") as expps_pool:
                exp_tiles = {}

                def expert_block(pos):
                    l = SWEEP[pos]
                    ps = expps_pool.tile([P, BC], f32, tag="expps", name="expps")
                    for j in range(NBK):
                        cs = slice(j * 512, (j + 1) * 512)
                        for k in range(KC):
                            nc.tensor.matmul(
                                ps[:, cs],
                                wall_t[k][:, pos * P : (pos + 1) * P],
                                xt_t[k][:, cs],
                                start=(k == 0),
                                stop=(k == KC - 1),
                            )
                    x_eb = exp_pool.tile([P, BC], f16, tag="xeb", name="xeb")
                    nc.scalar.activation(
                        x_eb[:],
                        ps[:],
                        mybir.ActivationFunctionType.Relu,
                        bias=biasT[:, l : l + 1],
                    )
                    exp_tiles[pos] = x_eb

                inited = set()

                def combine_block(pos):
                    l = SWEEP[pos]
                    expert, h = l // 2, l % 2
                    x_eb = exp_tiles.pop(pos)
                    for t, j in _uses(expert):
                        a = acc[t * 2 + h]
                        eng = nc.vector if t == 0 else nc.gpsimd
                        tp = tmpv_pool if t == 0 else tmpg_pool
                        if (t, h) not in inited:
                            inited.add((t, h))
                            eng.tensor_mul(a[:], x_eb[:], gbc[:, j, :])
                        else:
                            tmp = tp.tile([P, BC], f16, tag="tmp", name="tmp")
                            eng.tensor_mul(tmp[:], x_eb[:], gbc[:, j, :])
                            eng.tensor_add(a[:], a[:], tmp[:])

                for pos in range(NEB):
                    expert_block(pos)
                    combine_block(pos)

            # towers
            with (
                tc.tile_pool(name="hps", bufs=2, space="PSUM") as hps_pool,
                tc.tile_pool(name="ops", bufs=2, space="PSUM") as ops_pool,
            ):
                for t in range(T):
                    for bc in range(NBK):
                        cs = slice(bc * 512, (bc + 1) * 512)
                        hp = hps_pool.tile([P, 512], f32, tag="hps", name="hps")
                        for kc in range(2):
                            nc.tensor.matmul(
                                hp[:],
                                tw1_t[(t, kc)][:],
                                acc[t * 2 + kc][:, cs],
                                start=(kc == 0),
                                stop=(kc == 1),
                            )
                        hs = hsb_pool.tile([P, 512], f16, tag="hsb", name="hsb")
                        nc.scalar.activation(
                            hs[:],
                            hp[:],
                            mybir.ActivationFunctionType.Relu,
                            bias=tb1[:, t : t + 1],
                        )
                        op = ops_pool.tile([1, 512], f32, tag="ops", name="ops")
                        nc.tensor.matmul(
                            op[:],
                            tw2[:, t : t + 1],
                            hs[:],
                            start=True,
                            stop=True,
                        )
                        r = t * NBK + bc
                        nc.vector.tensor_copy(
                            out_sb[0:1, r * 512 : (r + 1) * 512], op[0:1, :]
                        )
                        nc.sync.dma_start(
                            out_d.rearrange("t n -> (t n)")[
                                None, r * 512 : (r + 1) * 512
                            ],
                            out_sb[0:1, r * 512 : (r + 1) * 512],
                        )

    nc.compile()
    return nc


_NC = None


def _get_nc():
    global _NC
    if _NC is None:
        _NC = _build()
    return _NC


def _prep_shared(shared_W, shared_b, task_W, task_b, gate_W, tower_W1, tower_b1, tower_W2):
    # expert columns in logical order: shared 0..3, task (t, k)
    cols = [np.asarray(shared_W[s]) for s in range(S)]
    cols += [np.asarray(task_W[t, k]) for t in range(T) for k in range(K)]
    ecols = np.concatenate(cols, axis=1)  # [D, 3072], col c = expert*256 + e
    # rearrange 128-col blocks into sweep order
    blocks = ecols.reshape(D, NE * 2, P)
    swept = blocks[:, SWEEP, :].reshape(D, WCOLS)
    gwi = np.empty((D, TG), np.float32)
    for t in range(T):
        gwi[:, t * G : (t + 1) * G] = np.asarray(gate_W[t])  # col t*8+g
    wall = np.ascontiguousarray(
        np.concatenate([swept, gwi], axis=1), dtype=np.float16
    )
    bias_all = np.concatenate(
        [np.asarray(shared_b).reshape(-1), np.asarray(task_b).reshape(-1)]
    ).astype(np.float32)
    biasT = np.ascontiguousarray(bias_all.reshape(NEB, P).T)  # [128, 24], col = logical eb
    tw1 = np.ascontiguousarray(tower_W1, dtype=np.float16)
    tb1 = np.ascontiguousarray(np.asarray(tower_b1).T, dtype=np.float32)   # [H, T]
    tw2 = np.ascontiguousarray(np.asarray(tower_W2)[:, :, 0].T, dtype=np.float16)  # [H, T]
    return wall, biasT, tw1, tb1, tw2


def kernel(
    x,
    shared_W,
    shared_b,
    task_W,
    task_b,
    gate_W,
    tower_W1,
    tower_b1,
    tower_W2,
    tower_b2,
    _trace=False,
    _tmpdir=None,
):
    nc = _get_nc()
    x = np.asarray(x, dtype=np.float32)
    wall, biasT, tw1, tb1, tw2 = _prep_shared(
        shared_W, shared_b, task_W, task_b, gate_W, tower_W1, tower_b1, tower_W2
    )
    in_maps = []
    for c in range(NCORES):
        xt = np.ascontiguousarray(x[c * BC : (c + 1) * BC, :].T.astype(np.float16))
        in_maps.append(
            {
                "xt": xt,
                "wall": wall,
                "biasT": biasT,
                "tw1": tw1,
                "tb1": tb1,
                "tw2": tw2,
            }
        )
    kw = {}
    if _trace:
        kw = {"trace": True, "tmpdir": _tmpdir}
    res = run_bass_kernel_spmd(nc, in_maps, core_ids=list(range(NCORES)), **kw)
    out = np.concatenate([res.results[c]["out"] for c in range(NCORES)], axis=1)
    out = out + np.asarray(tower_b2, dtype=np.float32)[:, 0][:, None]
    result = out[:, :, None].astype(np.float32)  # [T, B, 1]
    if _trace:
        return result, res
    return result


# revision 7
# speedup vs baseline: 1.3984x; 1.3984x over previous
"""MMoE-style CustomizedGateControl kernel for 8x TRN2 NeuronCores.

Data-parallel over the batch dim (16384 -> 8 x 2048). Per core, expert
GEMMs run weights-stationary streaming batch columns, producing outputs
directly in [e, b] layout:
  - 12 expert GEMMs as 24 column-blocks (eb) of 128 e-rows x 2048 b-cols,
    PSUM [128, 2048] (4 banks) double-buffered, k-outer loop order so the
    walrus ldw-opt pass can elide redundant weight loads
  - drain = ONE fused ACT op per eb: relu(psum + bias[e]) -> fp16 SBUF
    (bias is per-partition in this layout)
  - gates [16, 2048] via tiny PE GEMM, broadcast to [128, 16, 2048] with
    partition-replicating DMA reads from a DRAM bounce buffer
  - gated combine on DVE only (GpSimd elementwise would mutually block
    with DVE on the shared SBUF port pair); shared experts feed both
    tasks in one paired [128, 2, 2048] op
  - tower MLPs read the [128, T, 2048] accumulators directly
All parameters replicated; no collectives.
"""

import sys

if "/opt/trn_rl_repo" not in sys.path:
    sys.path.insert(0, "/opt/trn_rl_repo")

import numpy as np

import concourse.bacc as bacc
import concourse.mybir as mybir
import concourse.tile as tile
import concourse.bass_utils as _bu
from concourse.bass_utils import run_bass_kernel_spmd

# Enable the walrus LDWEIGHTS-elision pass (off by default); fall back to
# the original flags if the pass rejects this kernel.
if not getattr(_bu, "_ldw_patched", False):
    _orig_rc = _bu.run_command

    def _rc(cmd, *a, **k):
        cmd2 = [
            ("--enable-ldw-opt=true" if c == "--enable-ldw-opt=false" else c)
            for c in cmd
        ]
        if cmd2 != cmd:
            try:
                return _orig_rc(cmd2, *a, **k)
            except Exception:
                pass
        return _orig_rc(cmd, *a, **k)

    _bu.run_command = _rc
    _bu._ldw_patched = True

# problem dims
B, D, E, H = 16384, 512, 256, 128
S, K, T = 4, 4, 2
NCORES = 8
BC = B // NCORES          # 2048 batch rows per core
P = 128                   # partitions
KC = D // P               # 4 contraction chunks
NE = S + T * K            # 12 experts
G = S + K                 # 8 gate inputs per task
TG = T * G                # 16 gate columns
NEB = NE * 2              # 24 expert-output blocks of 128 e-rows
WCOLS = NE * E            # 3072 expert output columns
WALL = WCOLS + TG         # 3088 = experts + gate columns
NBK = BC // 512           # 4 batch chunks of 512 (psum bank size)

f32 = mybir.dt.float32
f16 = mybir.dt.float16

# sweep order: all half-0 blocks first, then half-1 (logical eb = expert*2+h)
SWEEP = [e * 2 for e in range(NE)] + [e * 2 + 1 for e in range(NE)]


def _build():
    nc = bacc.Bacc("TRN2", target_bir_lowering=False, debug=False)

    xt_d = nc.dram_tensor("xt", [D, BC], f16, kind="ExternalInput").ap()
    wall_d = nc.dram_tensor("wall", [D, WALL], f16, kind="ExternalInput").ap()
    biasT_d = nc.dram_tensor("biasT", [P, NEB], f32, kind="ExternalInput").ap()
    tw1_d = nc.dram_tensor("tw1", [T, E, H], f16, kind="ExternalInput").ap()
    tb1_d = nc.dram_tensor("tb1", [H, T], f32, kind="ExternalInput").ap()
    tw2_d = nc.dram_tensor("tw2", [H, T], f16, kind="ExternalInput").ap()
    out_d = nc.dram_tensor("out", [T, BC], f32, kind="ExternalOutput").ap()
    gbounce_d = nc.dram_tensor("gbounce", [TG, BC], f16, kind="Internal").ap()

    with tile.TileContext(nc) as tc:
        with (
            tc.tile_pool(name="const", bufs=1) as const,
            tc.tile_pool(name="exp", bufs=6) as exp_pool,
            tc.tile_pool(name="tmpv", bufs=2) as tmpv_pool,
        ):
            xt_t = [const.tile([P, BC], f16, tag=f"xt{k}", name=f"xt{k}") for k in range(KC)]
            wall_t = [const.tile([P, WALL], f16, tag=f"wall{k}", name=f"wall{k}") for k in range(KC)]
            biasT = const.tile([P, NEB], f32, tag="biasT", name="biasT")
            gates_sb = const.tile([TG, BC], f16, tag="gates_sb", name="gates_sb")
            gbc = const.tile([P, TG, BC], f16, tag="gbc", name="gbc")
            # acc[h]: [128, T, BC] fp16, t-major so paired ops hit both tasks
            acc = [
                const.tile([P, T, BC], f16, tag=f"acc{h}", name=f"acc{h}") for h in range(2)
            ]
            tw1_t = {}
            for t in range(T):
                for kc in range(2):
                    t_ = const.tile([P, H], f16, tag=f"tw1_{t}_{kc}", name=f"tw1_{t}_{kc}")
                    tw1_t[(t, kc)] = t_
            tb1 = const.tile([H, T], f32, tag="tb1", name="tb1")
            tw2 = const.tile([H, T], f16, tag="tw2", name="tw2")
            hs_t = [
                const.tile([P, BC], f16, tag=f"hs{t}", name=f"hs{t}") for t in range(T)
            ]
            out_sb = const.tile([1, T * BC], f32, tag="out_sb", name="out_sb")

            # ---- input DMAs, b-chunked so compute can start early ----
            # sync: xt k=0,1 j-chunked; scalar: xt k=2,3 halved
            for j in range(NBK):
                cs = slice(j * 512, (j + 1) * 512)
                nc.sync.dma_start(xt_t[0][:, cs], xt_d[0 * P : 1 * P, cs])
                nc.sync.dma_start(xt_t[1][:, cs], xt_d[1 * P : 2 * P, cs])
            for half in range(2):
                cs = slice(half * 1024, (half + 1) * 1024)
                nc.scalar.dma_start(xt_t[2][:, cs], xt_d[2 * P : 3 * P, cs])
                nc.scalar.dma_start(xt_t[3][:, cs], xt_d[3 * P : 4 * P, cs])
            # gpsimd: gate weight cols first, then wall halves in sweep order
            for k in range(KC):
                rs = slice(k * P, (k + 1) * P)
                nc.gpsimd.dma_start(wall_t[k][:, WCOLS:WALL], wall_d[rs, WCOLS:WALL])
            HALF = NEB // 2 * P  # 1536 cols per half
            for k in range(KC):
                rs = slice(k * P, (k + 1) * P)
                nc.gpsimd.dma_start(wall_t[k][:, 0:HALF], wall_d[rs, 0:HALF])
            for k in range(KC):
                rs = slice(k * P, (k + 1) * P)
                nc.gpsimd.dma_start(wall_t[k][:, HALF:WCOLS], wall_d[rs, HALF:WCOLS])
            # small consts at the back of the sync queue (needed late)
            nc.sync.dma_start(biasT[:], biasT_d[:])
            nc.sync.dma_start(tb1[:], tb1_d[:])
            nc.sync.dma_start(tw2[:], tw2_d[:])
            for t in range(T):
                for kc in range(2):
                    nc.sync.dma_start(
                        tw1_t[(t, kc)][:], tw1_d[t, kc * P : (kc + 1) * P, :]
                    )

            with tc.tile_pool(name="expps", bufs=2, space="PSUM") as expps_pool:
                exp_tiles = {}

                def expert_block(pos):
                    l = SWEEP[pos]
                    ps = expps_pool.tile([P, BC], f32, tag="expps", name="expps")
                    for k in range(KC):
                        for j in range(NBK):
                            cs = slice(j * 512, (j + 1) * 512)
                            nc.tensor.matmul(
                                ps[:, cs],
                                wall_t[k][:, pos * P : (pos + 1) * P],
                                xt_t[k][:, cs],
                                start=(k == 0),
                                stop=(k == KC - 1),
                                skip_group_check=True,
                            )
                    x_eb = exp_pool.tile([P, BC], f16, tag="xeb", name="xeb")
                    nc.scalar.activation(
                        x_eb[:],
                        ps[:],
                        mybir.ActivationFunctionType.Relu,
                        bias=biasT[:, l : l + 1],
                    )
                    exp_tiles[pos] = x_eb

                # first expert block warms PE while the rest of xt streams in
                expert_block(0)

                # gates in rows 0:16 of a psum-pool tile (k-outer for ldw-opt)
                gate_ps = expps_pool.tile([P, BC], f32, tag="expps", name="gate_ps")
                for k in range(KC):
                    for j in range(NBK):
                        cs = slice(j * 512, (j + 1) * 512)
                        nc.tensor.matmul(
                            gate_ps[0:TG, cs],
                            wall_t[k][:, WCOLS:WALL],
                            xt_t[k][:, cs],
                            start=(k == 0),
                            stop=(k == KC - 1),
                            skip_group_check=True,
                        )
                nc.scalar.copy(gates_sb[:], gate_ps[0:TG, :])

                # bounce gates through DRAM with partition-replicating reads.
                # Each queue writes its own row-slice then reads it back
                # (same-queue FIFO ordering guarantees write-before-read).
                row_groups = [
                    (nc.sync, [0, 8, 1, 9, 2, 10]),
                    (nc.scalar, [3, 11, 4, 5]),
                    (nc.gpsimd, [6, 7, 12, 13, 14, 15]),
                ]
                for eng, rows in row_groups:
                    for r in rows:
                        eng.dma_start(gbounce_d[r : r + 1, :], gates_sb[r : r + 1, :])
                    for r in rows:
                        eng.dma_start(
                            gbc[:, r, :], gbounce_d[r : r + 1, :].broadcast_to([P, BC])
                        )

                inited = set()

                def combine_block(pos):
                    l = SWEEP[pos]
                    expert, h = l // 2, l % 2
                    x_eb = exp_tiles.pop(pos)
                    a = acc[h]
                    if expert < S:
                        # shared expert: both tasks in one paired op
                        g2 = gbc[:, expert : expert + G + 1 : G, :]  # rows (g, 8+g)
                        xb = x_eb[:, None, :].broadcast_to([P, T, BC])
                        if ("s", h) not in inited:
                            inited.add(("s", h))
                            nc.vector.tensor_mul(a[:], xb, g2)
                        else:
                            tmp = tmpv_pool.tile([P, T, BC], f16, tag="tmp", name="tmp")
                            nc.vector.tensor_mul(tmp[:], xb, g2)
                            nc.vector.tensor_add(a[:], a[:], tmp[:])
                    else:
                        t = (expert - S) // K
                        j = t * G + S + (expert - S) % K
                        tmp = tmpv_pool.tile([P, T, BC], f16, tag="tmp", name="tmp")
                        nc.vector.tensor_mul(tmp[:, 0, :], x_eb[:], gbc[:, j, :])
                        nc.vector.tensor_add(a[:, t, :], a[:, t, :], tmp[:, 0, :])

                for pos in range(NEB):
                    if pos > 0:
                        expert_block(pos)
                    combine_block(pos)

                # towers: psum tiles from the same pool (experts are done)
                for t in range(T):
                    hp = expps_pool.tile([P, BC], f32, tag="expps", name=f"hp{t}")
                    for kc in range(2):
                        for j in range(NBK):
                            cs = slice(j * 512, (j + 1) * 512)
                            nc.tensor.matmul(
                                hp[:, cs],
                                tw1_t[(t, kc)][:],
                                acc[kc][:, t, cs],
                                start=(kc == 0),
                                stop=(kc == 1),
                                skip_group_check=True,
                            )
                    nc.scalar.activation(
                        hs_t[t][:],
                        hp[:],
                        mybir.ActivationFunctionType.Relu,
                        bias=tb1[:, t : t + 1],
                    )
                    op = expps_pool.tile([P, BC], f32, tag="expps", name=f"op{t}")
                    for j in range(NBK):
                        cs = slice(j * 512, (j + 1) * 512)
                        nc.tensor.matmul(
                            op[0:1, cs],
                            tw2[:, t : t + 1],
                            hs_t[t][:, cs],
                            start=True,
                            stop=True,
                            skip_group_check=True,
                        )
                    nc.vector.tensor_copy(
                        out_sb[0:1, t * BC : (t + 1) * BC], op[0:1, :]
                    )
                    nc.sync.dma_start(
                        out_d[t : t + 1, :], out_sb[0:1, t * BC : (t + 1) * BC]
                    )

    nc.compile()
    return nc


_NC = None


def _get_nc():
    global _NC
    if _NC is None:
        _NC = _build()
    return _NC


def _prep_shared(shared_W, shared_b, task_W, task_b, gate_W, tower_W1, tower_b1, tower_W2):
    # expert columns in logical order: shared 0..3, task (t, k)
    cols = [np.asarray(shared_W[s]) for s in range(S)]
    cols += [np.asarray(task_W[t, k]) for t in range(T) for k in range(K)]
    ecols = np.concatenate(cols, axis=1)  # [D, 3072], col c = expert*256 + e
    # rearrange 128-col blocks into sweep order
    blocks = ecols.reshape(D, NE * 2, P)
    swept = blocks[:, SWEEP, :].reshape(D, WCOLS)
    gwi = np.empty((D, TG), np.float32)
    for t in range(T):
        gwi[:, t * G : (t + 1) * G] = np.asarray(gate_W[t])  # col t*8+g
    wall = np.ascontiguousarray(
        np.concatenate([swept, gwi], axis=1), dtype=np.float16
    )
    bias_all = np.concatenate(
        [np.asarray(shared_b).reshape(-1), np.asarray(task_b).reshape(-1)]
    ).astype(np.float32)
    biasT = np.ascontiguousarray(bias_all.reshape(NEB, P).T)  # [128, 24], col = logical eb
    tw1 = np.ascontiguousarray(tower_W1, dtype=np.float16)
    tb1 = np.ascontiguousarray(np.asarray(tower_b1).T, dtype=np.float32)   # [H, T]
    tw2 = np.ascontiguousarray(np.asarray(tower_W2)[:, :, 0].T, dtype=np.float16)  # [H, T]
    return wall, biasT, tw1, tb1, tw2


def kernel(
    x,
    shared_W,
    shared_b,
    task_W,
    task_b,
    gate_W,
    tower_W1,
    tower_b1,
    tower_W2,
    tower_b2,
    _trace=False,
    _tmpdir=None,
):
    nc = _get_nc()
    x = np.asarray(x, dtype=np.float32)
    wall, biasT, tw1, tb1, tw2 = _prep_shared(
        shared_W, shared_b, task_W, task_b, gate_W, tower_W1, tower_b1, tower_W2
    )
    in_maps = []
    for c in range(NCORES):
        xt = np.ascontiguousarray(x[c * BC : (c + 1) * BC, :].T.astype(np.float16))
        in_maps.append(
            {
                "xt": xt,
                "wall": wall,
                "biasT": biasT,
                "tw1": tw1,
                "tb1": tb1,
                "tw2": tw2,
            }
        )
    kw = {}
    if _trace:
        kw = {"trace": True, "tmpdir": _tmpdir}
    res = run_bass_kernel_spmd(nc, in_maps, core_ids=list(range(NCORES)), **kw)
    out = np.concatenate([res.results[c]["out"] for c in range(NCORES)], axis=1)
    out = out + np.asarray(tower_b2, dtype=np.float32)[:, 0][:, None]
    result = out[:, :, None].astype(np.float32)  # [T, B, 1]
    if _trace:
        return result, res
    return result


# revision 13
# speedup vs baseline: 1.5182x; 1.0857x over previous
"""MMoE-style CustomizedGateControl kernel for 8x TRN2 NeuronCores.

Data-parallel over the batch dim (16384 -> 8 x 2048). Per core, expert
GEMMs run weights-stationary streaming batch columns, producing outputs
directly in [e, b] layout:
  - 12 expert GEMMs as 24 column-blocks (eb) of 128 e-rows x 2048 b-cols,
    PSUM [128, 2048] (4 banks) double-buffered, k-outer loop order so the
    walrus ldw-opt pass can elide redundant weight loads
  - drain = ONE fused ACT op per eb: relu(psum + bias[e]) -> fp16 SBUF
    (bias is per-partition in this layout)
  - gates [16, 2048] via tiny PE GEMM, broadcast to [128, 16, 2048] with
    partition-replicating DMA reads from a DRAM bounce buffer
  - gated combine on DVE only (GpSimd elementwise would mutually block
    with DVE on the shared SBUF port pair); shared experts feed both
    tasks in one paired [128, 2, 2048] op
  - tower MLPs read the [128, T, 2048] accumulators directly
All parameters replicated; no collectives.
"""

import sys

if "/opt/trn_rl_repo" not in sys.path:
    sys.path.insert(0, "/opt/trn_rl_repo")

import numpy as np

import concourse.bacc as bacc
import concourse.mybir as mybir
import concourse.tile as tile
import concourse.bass_utils as _bu
from concourse.bass_utils import run_bass_kernel_spmd

# problem dims
B, D, E, H = 16384, 512, 256, 128
S, K, T = 4, 4, 2
NCORES = 8
BC = B // NCORES          # 2048 batch rows per core
P = 128                   # partitions
KC = D // P               # 4 contraction chunks
NE = S + T * K            # 12 experts
G = S + K                 # 8 gate inputs per task
TG = T * G                # 16 gate columns
NEB = NE * 2              # 24 expert-output blocks of 128 e-rows
WCOLS = NE * E            # 3072 expert output columns
WALL = WCOLS + TG         # 3088 = experts + gate columns
NBK = BC // 512           # 4 batch chunks of 512 (psum bank size)

f32 = mybir.dt.float32
f16 = mybir.dt.float16

# sweep order: all half-0 blocks first, then half-1 (logical eb = expert*2+h)
SWEEP = [e * 2 for e in range(NE)] + [e * 2 + 1 for e in range(NE)]


def _build():
    nc = bacc.Bacc("TRN2", target_bir_lowering=False, debug=False)

    xt_d = nc.dram_tensor("xt", [D, BC], f16, kind="ExternalInput").ap()
    wall_d = nc.dram_tensor("wall", [D, WALL], f16, kind="ExternalInput").ap()
    biasT_d = nc.dram_tensor("biasT", [P, NEB], f32, kind="ExternalInput").ap()
    tw1_d = nc.dram_tensor("tw1", [T, E, H], f16, kind="ExternalInput").ap()
    tb1_d = nc.dram_tensor("tb1", [H, T], f32, kind="ExternalInput").ap()
    tw2_d = nc.dram_tensor("tw2", [H, T], f16, kind="ExternalInput").ap()
    out_d = nc.dram_tensor("out", [T, BC], f32, kind="ExternalOutput").ap()
    gbounce_d = nc.dram_tensor("gbounce", [TG, BC], f16, kind="Internal").ap()

    with tile.TileContext(nc) as tc:
        with (
            tc.tile_pool(name="const", bufs=1) as const,
            tc.tile_pool(name="exp", bufs=7) as exp_pool,
            tc.tile_pool(name="tmpv", bufs=2) as tmpv_pool,
        ):
            xt_t = [const.tile([P, BC], f16, tag=f"xt{k}", name=f"xt{k}") for k in range(KC)]
            wall_t = [const.tile([P, WALL], f16, tag=f"wall{k}", name=f"wall{k}") for k in range(KC)]
            biasT = const.tile([P, NEB], f32, tag="biasT", name="biasT")
            gates_sb = const.tile([TG, BC], f16, tag="gates_sb", name="gates_sb")
            gbc = const.tile([P, TG, BC], f16, tag="gbc", name="gbc")
            # acc[h]: [128, T, BC] fp16, t-major so paired ops hit both tasks
            acc = [
                const.tile([P, T, BC], f16, tag=f"acc{h}", name=f"acc{h}") for h in range(2)
            ]
            tw1_t = {}
            for t in range(T):
                for kc in range(2):
                    t_ = const.tile([P, H], f16, tag=f"tw1_{t}_{kc}", name=f"tw1_{t}_{kc}")
                    tw1_t[(t, kc)] = t_
            tb1 = const.tile([H, T], f32, tag="tb1", name="tb1")
            tw2 = const.tile([H, T], f16, tag="tw2", name="tw2")
            hs_t = [
                const.tile([P, BC], f16, tag=f"hs{t}", name=f"hs{t}") for t in range(T)
            ]
            out_sb = const.tile([1, T * BC], f32, tag="out_sb", name="out_sb")

            # ---- input DMAs on the two fast HWDGE queues (sync/scalar),
            # head chunks first so eb0/eb1 can start within ~2us. The slow
            # SWDGE gpsimd queue only carries late-needed bounce rows.
            HALF = NEB // 2 * P  # 1536 cols per half
            RW = {k: slice(k * P, (k + 1) * P) for k in range(KC)}
            # heads: first batch chunk of xt + first two weight blocks
            nc.sync.dma_start(xt_t[0][:, 0:512], xt_d[RW[0], 0:512])
            nc.scalar.dma_start(xt_t[2][:, 0:512], xt_d[RW[2], 0:512])
            nc.sync.dma_start(xt_t[1][:, 0:512], xt_d[RW[1], 0:512])
            nc.scalar.dma_start(xt_t[3][:, 0:512], xt_d[RW[3], 0:512])
            nc.sync.dma_start(wall_t[0][:, 0:256], wall_d[RW[0], 0:256])
            nc.scalar.dma_start(wall_t[2][:, 0:256], wall_d[RW[2], 0:256])
            nc.sync.dma_start(wall_t[1][:, 0:256], wall_d[RW[1], 0:256])
            nc.scalar.dma_start(wall_t[3][:, 0:256], wall_d[RW[3], 0:256])
            # gate weights + bias (tiny, needed early)
            for k in range(KC):
                nc.sync.dma_start(wall_t[k][:, WCOLS:WALL], wall_d[RW[k], WCOLS:WALL])
            nc.sync.dma_start(biasT[:], biasT_d[:])
            # rest of xt
            nc.sync.dma_start(xt_t[0][:, 512:BC], xt_d[RW[0], 512:BC])
            nc.scalar.dma_start(xt_t[2][:, 512:BC], xt_d[RW[2], 512:BC])
            nc.sync.dma_start(xt_t[1][:, 512:BC], xt_d[RW[1], 512:BC])
            nc.scalar.dma_start(xt_t[3][:, 512:BC], xt_d[RW[3], 512:BC])
            # rest of the first-half weights
            nc.sync.dma_start(wall_t[0][:, 256:HALF], wall_d[RW[0], 256:HALF])
            nc.scalar.dma_start(wall_t[2][:, 256:HALF], wall_d[RW[2], 256:HALF])
            nc.sync.dma_start(wall_t[1][:, 256:HALF], wall_d[RW[1], 256:HALF])
            nc.scalar.dma_start(wall_t[3][:, 256:HALF], wall_d[RW[3], 256:HALF])

            with tc.tile_pool(name="expps", bufs=2, space="PSUM") as expps_pool:
                exp_tiles = {}

                def expert_block(pos):
                    l = SWEEP[pos]
                    ps = expps_pool.tile([P, BC], f32, tag="expps", name="expps")
                    for k in range(KC):
                        for j in range(NBK):
                            cs = slice(j * 512, (j + 1) * 512)
                            nc.tensor.matmul(
                                ps[:, cs],
                                wall_t[k][:, pos * P : (pos + 1) * P],
                                xt_t[k][:, cs],
                                start=(k == 0),
                                stop=(k == KC - 1),
                                skip_group_check=True,
                            )
                    x_eb = exp_pool.tile([P, BC], f16, tag="xeb", name="xeb")
                    nc.scalar.activation(
                        x_eb[:],
                        ps[:],
                        mybir.ActivationFunctionType.Relu,
                        bias=biasT[:, l : l + 1],
                    )
                    exp_tiles[pos] = x_eb

                # first expert blocks warm PE while the rest of xt streams in
                expert_block(0)
                expert_block(1)

                # gates in rows 0:16 of a psum-pool tile (k-outer for ldw-opt)
                gate_ps = expps_pool.tile([P, BC], f32, tag="expps", name="gate_ps")
                for k in range(KC):
                    for j in range(NBK):
                        cs = slice(j * 512, (j + 1) * 512)
                        nc.tensor.matmul(
                            gate_ps[0:TG, cs],
                            wall_t[k][:, WCOLS:WALL],
                            xt_t[k][:, cs],
                            start=(k == 0),
                            stop=(k == KC - 1),
                            skip_group_check=True,
                        )
                nc.scalar.copy(gates_sb[:], gate_ps[0:TG, :])

                # bounce gates through DRAM with partition-replicating reads.
                # Each queue writes its own row-slice then reads it back
                # (same-queue FIFO ordering guarantees write-before-read).
                # Rows ordered by first consumption in the sweep.
                row_groups = [
                    (nc.sync, [0, 8, 1, 9]),
                    (nc.scalar, [2, 10, 3, 11]),
                    (nc.gpsimd, [4, 5, 6, 7, 12, 13, 14, 15]),
                ]
                for eng, rows in row_groups:
                    for r in rows:
                        eng.dma_start(gbounce_d[r : r + 1, :], gates_sb[r : r + 1, :])
                    for r in rows:
                        eng.dma_start(
                            gbc[:, r, :], gbounce_d[r : r + 1, :].broadcast_to([P, BC])
                        )

                # second-half weights (needed from sweep pos 12) and tower
                # consts, queued behind the bounce traffic
                nc.sync.dma_start(wall_t[0][:, HALF:WCOLS], wall_d[RW[0], HALF:WCOLS])
                nc.scalar.dma_start(wall_t[2][:, HALF:WCOLS], wall_d[RW[2], HALF:WCOLS])
                nc.sync.dma_start(wall_t[1][:, HALF:WCOLS], wall_d[RW[1], HALF:WCOLS])
                nc.scalar.dma_start(wall_t[3][:, HALF:WCOLS], wall_d[RW[3], HALF:WCOLS])
                nc.sync.dma_start(tb1[:], tb1_d[:])
                nc.sync.dma_start(tw2[:], tw2_d[:])
                for t in range(T):
                    for kc in range(2):
                        nc.sync.dma_start(
                            tw1_t[(t, kc)][:], tw1_d[t, kc * P : (kc + 1) * P, :]
                        )

                inited = set()

                def combine_block(pos):
                    l = SWEEP[pos]
                    expert, h = l // 2, l % 2
                    x_eb = exp_tiles.pop(pos)
                    a = acc[h]
                    if expert < S:
                        # shared expert: both tasks in one paired op
                        g2 = gbc[:, expert : expert + G + 1 : G, :]  # rows (g, 8+g)
                        xb = x_eb[:, None, :].broadcast_to([P, T, BC])
                        if ("s", h) not in inited:
                            inited.add(("s", h))
                            nc.vector.tensor_mul(a[:], xb, g2)
                        else:
                            tmp = tmpv_pool.tile([P, T, BC], f16, tag="tmp", name="tmp")
                            nc.vector.tensor_mul(tmp[:], xb, g2)
                            nc.vector.tensor_add(a[:], a[:], tmp[:])
                    else:
                        t = (expert - S) // K
                        j = t * G + S + (expert - S) % K
                        tmp = tmpv_pool.tile([P, T, BC], f16, tag="tmp", name="tmp")
                        nc.vector.tensor_mul(tmp[:, 0, :], x_eb[:], gbc[:, j, :])
                        nc.vector.tensor_add(a[:, t, :], a[:, t, :], tmp[:, 0, :])

                for pos in range(NEB):
                    if pos > 1:
                        expert_block(pos)
                    combine_block(pos)

                # towers: psum tiles from the same pool (experts are done)
                for t in range(T):
                    hp = expps_pool.tile([P, BC], f32, tag="expps", name=f"hp{t}")
                    for kc in range(2):
                        for j in range(NBK):
                            cs = slice(j * 512, (j + 1) * 512)
                            nc.tensor.matmul(
                                hp[:, cs],
                                tw1_t[(t, kc)][:],
                                acc[kc][:, t, cs],
                                start=(kc == 0),
                                stop=(kc == 1),
                                skip_group_check=True,
                            )
                    nc.scalar.activation(
                        hs_t[t][:],
                        hp[:],
                        mybir.ActivationFunctionType.Relu,
                        bias=tb1[:, t : t + 1],
                    )
                    op = expps_pool.tile([P, BC], f32, tag="expps", name=f"op{t}")
                    for j in range(NBK):
                        cs = slice(j * 512, (j + 1) * 512)
                        nc.tensor.matmul(
                            op[0:1, cs],
                            tw2[:, t : t + 1],
                            hs_t[t][:, cs],
                            start=True,
                            stop=True,
                            skip_group_check=True,
                        )
                    nc.vector.tensor_copy(
                        out_sb[0:1, t * BC : (t + 1) * BC], op[0:1, :]
                    )
                    nc.sync.dma_start(
                        out_d[t : t + 1, :], out_sb[0:1, t * BC : (t + 1) * BC]
                    )

    nc.compile()
    return nc


_NC = None


def _get_nc():
    global _NC
    if _NC is None:
        _NC = _build()
    return _NC


def _prep_shared(shared_W, shared_b, task_W, task_b, gate_W, tower_W1, tower_b1, tower_W2):
    # expert columns in logical order: shared 0..3, task (t, k)
    cols = [np.asarray(shared_W[s]) for s in range(S)]
    cols += [np.asarray(task_W[t, k]) for t in range(T) for k in range(K)]
    ecols = np.concatenate(cols, axis=1)  # [D, 3072], col c = expert*256 + e
    # rearrange 128-col blocks into sweep order
    blocks = ecols.reshape(D, NE * 2, P)
    swept = blocks[:, SWEEP, :].reshape(D, WCOLS)
    gwi = np.empty((D, TG), np.float32)
    for t in range(T):
        gwi[:, t * G : (t + 1) * G] = np.asarray(gate_W[t])  # col t*8+g
    wall = np.ascontiguousarray(
        np.concatenate([swept, gwi], axis=1), dtype=np.float16
    )
    bias_all = np.concatenate(
        [np.asarray(shared_b).reshape(-1), np.asarray(task_b).reshape(-1)]
    ).astype(np.float32)
    biasT = np.ascontiguousarray(bias_all.reshape(NEB, P).T)  # [128, 24], col = logical eb
    tw1 = np.ascontiguousarray(tower_W1, dtype=np.float16)
    tb1 = np.ascontiguousarray(np.asarray(tower_b1).T, dtype=np.float32)   # [H, T]
    tw2 = np.ascontiguousarray(np.asarray(tower_W2)[:, :, 0].T, dtype=np.float16)  # [H, T]
    return wall, biasT, tw1, tb1, tw2


def kernel(
    x,
    shared_W,
    shared_b,
    task_W,
    task_b,
    gate_W,
    tower_W1,
    tower_b1,
    tower_W2,
    tower_b2,
    _trace=False,
    _tmpdir=None,
):
    nc = _get_nc()
    x = np.asarray(x, dtype=np.float32)
    wall, biasT, tw1, tb1, tw2 = _prep_shared(
        shared_W, shared_b, task_W, task_b, gate_W, tower_W1, tower_b1, tower_W2
    )
    in_maps = []
    for c in range(NCORES):
        xt = np.ascontiguousarray(x[c * BC : (c + 1) * BC, :].T.astype(np.float16))
        in_maps.append(
            {
                "xt": xt,
                "wall": wall,
                "biasT": biasT,
                "tw1": tw1,
                "tb1": tb1,
                "tw2": tw2,
            }
        )
    kw = {}
    if _trace:
        kw = {"trace": True, "tmpdir": _tmpdir}
    res = run_bass_kernel_spmd(nc, in_maps, core_ids=list(range(NCORES)), **kw)
    out = np.concatenate([res.results[c]["out"] for c in range(NCORES)], axis=1)
    out = out + np.asarray(tower_b2, dtype=np.float32)[:, 0][:, None]
    result = out[:, :, None].astype(np.float32)  # [T, B, 1]
    if _trace:
        return result, res
    return result
